# revision 11
# baseline (speedup 1.0000x reference)
"""Spatial self-attention scores kernel for Trainium2 (8 NeuronCores).

Computes, per batch b:
    qk = W @ x_b          # [256, 4096] = [256,256] @ [256,4096]
    q, k = qk[:128], qk[128:]
    sim = (q.T @ k) * 128**-0.5
    out_b = softmax(sim, axis=-1)        # [4096, 4096]
Output: [8, 1, 4096, 4096] float32.

Sharding: data-parallel over batch, one batch image per NeuronCore.

Per-core pipeline (all phases overlap under the Tile scheduler):
  - x DMA'd in as fp16 (SWDGE cast); W transposed on PE via identity.
  - fp16 projection matmuls -> q,k in SBUF as [d=128, s=4096] fp16,
    interleaved with the first attention groups so the in-order PE
    reaches the first output as early as possible.
  - per 128-query row-tile: 8 fp16 matmuls (K=128, N=512) into 4-bank
    PSUM tiles; one ScalarE ACTIVATE per 2048 columns computes
    exp(SCALE*sim) with a fused row-sum (accum_out), writing fp16; DVE
    combines the partial sums, takes the reciprocal, and scales the row
    (fp16, 4x DVE mode).
  - output rows leave as fp16 in 2 MB DMAs (two row-tiles per transfer;
    the first group ships per normalized half-row); the host widens to
    fp32. ScalarE's exp stream (~64 x 2.3us) is the critical path.
"""

import numpy as np
from contextlib import ExitStack

import concourse.bass as bass
import concourse.tile as tile
from concourse import bacc, mybir
from concourse.bass_utils import run_bass_kernel_spmd
from concourse.masks import make_identity

B = 8
C = 256
HW = 4096
D = 128
SCALE = D ** -0.5
N_CORES = 8

BANK = 512             # PSUM bank width (fp32) = one matmul free-dim
ACT_CHUNK = 2048       # one ScalarE activation spans 4 banks
N_ACT = HW // ACT_CHUNK          # 2
GRP = 2                # row-tiles per output DMA (2 -> 2 MB transfers)
N_GRP = HW // (128 * GRP)        # 16
OUT_BUFS = 6
X_CHUNK = 1024         # x input DMA granularity (overlaps with projection)

F32 = mybir.dt.float32
# The whole pipeline runs in fp16: the PE streams fp16 at 1 cycle/row
# (2x float32r), DVE ops hit their 4x packed mode, and the output DMA
# moves half the bytes. The logits are ~N(0,1), so fp16's 10-bit
# mantissa keeps the end-to-end error ~1e-3 scale-relative -- well
# inside the 2e-2 gate. PSUM accumulation stays fp32.
MM_DT = mybir.dt.float16
PROJ_DT = mybir.dt.float16
OUT_DT = mybir.dt.float16


def _emit(ctx: ExitStack, tc: tile.TileContext, out_ap, x_ap, w_ap):
    nc = tc.nc

    const = ctx.enter_context(tc.tile_pool(name="const", bufs=1))
    data = ctx.enter_context(tc.tile_pool(name="data", bufs=1))
    psum = ctx.enter_context(tc.tile_pool(name="psum", bufs=2, space="PSUM"))
    small = ctx.enter_context(tc.tile_pool(name="small", bufs=4))

    # ---- x input DMAs issued before anything else: the SWDGE queue is
    # in-order, and everything downstream waits on these bytes. Each
    # chunk DMA casts fp32 -> fp16 on the fly and writes a contiguous
    # [chunk, c_tile, cols] block so projection banks depend only on
    # their own chunk's transfer.
    x_view = x_ap.rearrange("(t p) s -> p t s", p=128)
    x0_sb = data.tile([128, ACT_CHUNK // X_CHUNK, 2, X_CHUNK], PROJ_DT)
    x1_sb = data.tile([128, ACT_CHUNK // X_CHUNK, 2, X_CHUNK], PROJ_DT)
    for half, dst_x in ((0, x0_sb), (1, x1_sb)):
        for c in range(ACT_CHUNK // X_CHUNK):
            src = slice(half * ACT_CHUNK + c * X_CHUNK,
                        half * ACT_CHUNK + (c + 1) * X_CHUNK)
            nc.gpsimd.dma_start(out=dst_x[:, c], in_=x_view[:, :, src])

    # ---- W [256, 256] -> SBUF as [p, o_tile, c]
    w_sb = const.tile([128, 2, C], F32)
    nc.sync.dma_start(out=w_sb, in_=w_ap.rearrange("(t p) c -> p t c", p=128))

    # ---- PE warm-up: throwaway matmuls while x is loading. The PE
    # clock gate (HAM) only releases to 2.4 GHz after ~3.4 us of
    # sustained activity; warming during the input DMA makes the
    # projection and the first attention row-tiles run at full rate.
    warm_f32 = const.tile([128, BANK], F32)
    nc.vector.memset(warm_f32, 0.0)
    warm = const.tile([128, BANK], MM_DT)
    nc.vector.tensor_copy(out=warm, in_=warm_f32)
    wps = psum.tile([128, ACT_CHUNK], F32, tag="ps")
    for _ in range(10):
        nc.tensor.matmul(
            wps[:, 0:BANK], warm[:, 0:128], warm, start=True, stop=True
        )

    ident = const.tile([128, 128], F32)
    make_identity(nc, ident)

    # pull the exp table load off the first real activation
    tbl = small.tile([128, 1], F32, tag="tbl")
    nc.scalar.activation(
        out=tbl, in_=warm_f32[:, 0:1], func=mybir.ActivationFunctionType.Exp
    )

    # ---- transpose W on PE -> wt_sb[c_sub, c_tile, o] (contraction c on partitions)
    wt_sb = const.tile([128, 2, 2 * D], PROJ_DT)
    for t in range(2):          # output-channel tile (q half / k half)
        for ct in range(2):     # input-channel tile
            ps = psum.tile([128, ACT_CHUNK], F32, tag="ps")
            nc.tensor.transpose(
                ps[:, 0:128], w_sb[:, t, ct * 128:(ct + 1) * 128], ident
            )
            nc.vector.tensor_copy(
                out=wt_sb[:, ct, t * 128:(t + 1) * 128], in_=ps[:, 0:128]
            )

    q_sb = data.tile([128, HW], MM_DT)
    k_sb = data.tile([128, HW], MM_DT)

    def proj_chunk(t, dst, a, x_half, banks=None):
        """Project output-channel half t for column chunk a; x_half is
        [128, chunk, c_tile, X_CHUNK] holding x columns
        [a*ACT_CHUNK, (a+1)*ACT_CHUNK). banks selects a contiguous
        subset of the four 512-wide banks; the projected span leaves
        PSUM in a single batched DVE copy."""
        ps = psum.tile([128, ACT_CHUNK], F32, tag="ps")
        bank_list = list(banks) if banks is not None else list(
            range(ACT_CHUNK // BANK))
        for jj in bank_list:
            lo = slice(jj * BANK, (jj + 1) * BANK)
            ch = (jj * BANK) // X_CHUNK
            off = (jj * BANK) % X_CHUNK
            nc.tensor.matmul(
                ps[:, lo], wt_sb[:, 0, t * 128:(t + 1) * 128],
                x_half[:, ch, 0, off:off + BANK], start=True, stop=False,
            )
            nc.tensor.matmul(
                ps[:, lo], wt_sb[:, 1, t * 128:(t + 1) * 128],
                x_half[:, ch, 1, off:off + BANK], start=False, stop=True,
            )
        b0, b1 = bank_list[0], bank_list[-1] + 1
        nc.vector.tensor_copy(
            out=dst[:, a * ACT_CHUNK + b0 * BANK:a * ACT_CHUNK + b1 * BANK],
            in_=ps[:, b0 * BANK:b1 * BANK],
        )

    outp = None
    out_view = out_ap.rearrange("(g t p) m -> g p t m", t=GRP, p=128)

    def sim_chunk(lhs, out_row, lo_col, n_col, accum):
        """n_col-wide slice of one attention row: matmuls + fused exp."""
        ps = psum.tile([128, ACT_CHUNK], F32, tag="ps")
        for jj in range(n_col // BANK):
            sl = slice(lo_col + jj * BANK, lo_col + (jj + 1) * BANK)
            nc.tensor.matmul(
                ps[:, jj * BANK:(jj + 1) * BANK], lhs, k_sb[:, sl],
                start=True, stop=True,
            )
        nc.scalar.activation(
            out=out_row[:, lo_col:lo_col + n_col],
            in_=ps[:, 0:n_col],
            func=mybir.ActivationFunctionType.Exp,
            scale=SCALE,
            accum_out=accum,
        )

    def emit_group(g, split_dma=False):
        out_grp = outp.tile([128, GRP, HW], OUT_DT, tag="out")
        for t in range(GRP):
            i = g * GRP + t
            lhs = q_sb[:, i * 128:(i + 1) * 128]
            sums = small.tile([128, N_ACT], F32, tag="sums")
            for a in range(N_ACT):
                sim_chunk(lhs, out_grp[:, t], a * ACT_CHUNK, ACT_CHUNK,
                          sums[:, a:a + 1])
            rsum = small.tile([128, 1], F32, tag="rsum")
            nc.vector.tensor_reduce(
                out=rsum, in_=sums, axis=mybir.AxisListType.X,
                op=mybir.AluOpType.add,
            )
            recip = small.tile([128, 1], F32, tag="recip")
            nc.vector.reciprocal(out=recip, in_=rsum)
            if split_dma:
                # normalize and ship each half-row as soon as it is
                # scaled (1 MB transfers) so the first outputs leave at
                # the earliest possible moment
                i = g * GRP + t
                for a in range(N_ACT):
                    sl = slice(a * ACT_CHUNK, (a + 1) * ACT_CHUNK)
                    nc.vector.tensor_scalar_mul(
                        out=out_grp[:, t, sl], in0=out_grp[:, t, sl],
                        scalar1=recip,
                    )
                    nc.sync.dma_start(
                        out=out_ap[i * 128:(i + 1) * 128, sl],
                        in_=out_grp[:, t, sl],
                    )
            else:
                nc.vector.tensor_scalar_mul(
                    out=out_grp[:, t, :], in0=out_grp[:, t, :], scalar1=recip
                )
        if not split_dma:
            nc.sync.dma_start(out=out_view[g], in_=out_grp)

    # ---- projection: k fully projected before group 0 (every group's
    # second column chunk reads k[2048:4096]), with q's first bank
    # squeezed between the two k chunks so group 0's first sim matmuls
    # can issue as soon as k chunk 0 lands. Remaining q banks trickle in
    # one 512-wide bank at a time, each two groups ahead of its first
    # reader, so each PSUM borrow stays short.
    proj_chunk(1, k_sb, 0, x0_sb)               # k cols 0:2048
    proj_chunk(0, q_sb, 0, x0_sb, banks=(0,))   # q rows 0:512 (grps 0-1)
    proj_chunk(1, k_sb, 1, x1_sb)               # k cols 2048:4096

    outp = ctx.enter_context(tc.tile_pool(name="outp", bufs=OUT_BUFS))
    emit_group(0, split_dma=True)
    proj_chunk(0, q_sb, 0, x0_sb, banks=(1,))   # rows  512:1024 (grps 2-3)
    emit_group(1)
    proj_chunk(0, q_sb, 0, x0_sb, banks=(2,))   # rows 1024:1536 (grps 4-5)
    emit_group(2)
    proj_chunk(0, q_sb, 0, x0_sb, banks=(3,))   # rows 1536:2048 (grps 6-7)
    emit_group(3)
    for g in range(4, N_GRP // 2):
        # q chunk 1 (row-tiles 16-31), one bank two groups ahead
        proj_chunk(0, q_sb, 1, x1_sb, banks=(g - 4,))
        emit_group(g)
    for g in range(N_GRP // 2, N_GRP - 1):
        emit_group(g)
    # last group ships per normalized half-row so the tail after the
    # final exp is one 0.5 MB transfer, not a 2 MB one
    emit_group(N_GRP - 1, split_dma=True)


_built = None


def _get_nc():
    global _built
    if _built is None:
        nc = bacc.Bacc("TRN2", target_bir_lowering=False, debug=False)
        x = nc.dram_tensor("x", [C, HW], F32, kind="ExternalInput").ap()
        w = nc.dram_tensor("w", [2 * D, C], F32, kind="ExternalInput").ap()
        out = nc.dram_tensor("out", [HW, HW], OUT_DT, kind="ExternalOutput").ap()
        with tile.TileContext(nc) as tc:
            with ExitStack() as ctx:
                _emit(ctx, tc, out, x, w)
        nc.compile()
        _built = nc
    return _built


def kernel(x: np.ndarray, W: np.ndarray) -> np.ndarray:
    nc = _get_nc()
    x = np.asarray(x, dtype=np.float32)
    W = np.ascontiguousarray(np.asarray(W, dtype=np.float32))
    in_maps = [
        {"x": np.ascontiguousarray(x[b].reshape(C, HW)), "w": W} for b in range(B)
    ]
    res = run_bass_kernel_spmd(nc, in_maps, core_ids=list(range(N_CORES)))
    out = np.stack([res.results[b]["out"] for b in range(B)]).astype(np.float32)
    return out[:, None]



# revision 17
# speedup vs baseline: 1.0304x; 1.0304x over previous
"""Spatial self-attention scores kernel for Trainium2 (8 NeuronCores).

Computes, per batch b:
    qk = W @ x_b          # [256, 4096] = [256,256] @ [256,4096]
    q, k = qk[:128], qk[128:]
    sim = (q.T @ k) * 128**-0.5
    out_b = softmax(sim, axis=-1)        # [4096, 4096]
Output: [8, 1, 4096, 4096] float32.

Sharding: data-parallel over batch, one batch image per NeuronCore.

Per-core pipeline (all phases overlap under the Tile scheduler):
  - x DMA'd in as fp16 (SWDGE cast); W transposed on PE via identity.
  - fp16 projection matmuls -> q,k in SBUF as [d=128, s=4096] fp16,
    interleaved with the first attention groups so the in-order PE
    reaches the first output as early as possible.
  - per 128-query row-tile: 8 fp16 matmuls (K=128, N=512) into 4-bank
    PSUM tiles; one ScalarE ACTIVATE per 2048 columns computes
    exp(SCALE*sim) with a fused row-sum (accum_out), writing fp16; DVE
    combines the partial sums, takes the reciprocal, and scales the row
    (fp16, 4x DVE mode).
  - output rows leave as fp16 in 2 MB DMAs (two row-tiles per transfer;
    the first group ships per normalized half-row); the host widens to
    fp32. ScalarE's exp stream (~64 x 2.3us) is the critical path.
"""

import numpy as np
from contextlib import ExitStack

import concourse.bass as bass
import concourse.tile as tile
from concourse import bacc, mybir
from concourse.bass_utils import run_bass_kernel_spmd
from concourse.masks import make_identity

B = 8
C = 256
HW = 4096
D = 128
SCALE = D ** -0.5
N_CORES = 8

BANK = 512             # PSUM bank width (fp32) = one matmul free-dim
ACT_CHUNK = 2048       # one ScalarE activation spans 4 banks
N_ACT = HW // ACT_CHUNK          # 2
GRP = 2                # row-tiles per output DMA (2 -> 2 MB transfers)
N_GRP = HW // (128 * GRP)        # 16
OUT_BUFS = 6
X_CHUNK = 1024         # x input DMA granularity (overlaps with projection)

F32 = mybir.dt.float32
# The whole pipeline runs in fp16: the PE streams fp16 at 1 cycle/row
# (2x float32r), DVE ops hit their 4x packed mode, and the output DMA
# moves half the bytes. The logits are ~N(0,1), so fp16's 10-bit
# mantissa keeps the end-to-end error ~1e-3 scale-relative -- well
# inside the 2e-2 gate. PSUM accumulation stays fp32.
MM_DT = mybir.dt.float16
PROJ_DT = mybir.dt.float16
OUT_DT = mybir.dt.float16


def _emit(ctx: ExitStack, tc: tile.TileContext, out_ap, x_ap, w_ap):
    nc = tc.nc

    const = ctx.enter_context(tc.tile_pool(name="const", bufs=1))
    data = ctx.enter_context(tc.tile_pool(name="data", bufs=1))
    psum = ctx.enter_context(tc.tile_pool(name="psum", bufs=2, space="PSUM"))
    small = ctx.enter_context(tc.tile_pool(name="small", bufs=4))

    # ---- x input DMAs issued before anything else: the SWDGE queue is
    # in-order, and everything downstream waits on these bytes. Each
    # chunk DMA casts fp32 -> fp16 on the fly and writes a contiguous
    # [chunk, c_tile, cols] block so projection banks depend only on
    # their own chunk's transfer.
    x_view = x_ap.rearrange("(t p) s -> p t s", p=128)
    x0_sb = data.tile([128, ACT_CHUNK // X_CHUNK, 2, X_CHUNK], PROJ_DT)
    x1_sb = data.tile([128, ACT_CHUNK // X_CHUNK, 2, X_CHUNK], PROJ_DT)
    for half, dst_x in ((0, x0_sb), (1, x1_sb)):
        for c in range(ACT_CHUNK // X_CHUNK):
            src = slice(half * ACT_CHUNK + c * X_CHUNK,
                        half * ACT_CHUNK + (c + 1) * X_CHUNK)
            nc.gpsimd.dma_start(out=dst_x[:, c], in_=x_view[:, :, src])

    # ---- W [256, 256] -> SBUF as [p, o_tile, c]
    w_sb = const.tile([128, 2, C], F32)
    nc.sync.dma_start(out=w_sb, in_=w_ap.rearrange("(t p) c -> p t c", p=128))

    # ---- PE warm-up: throwaway matmuls while x is loading. The PE
    # clock gate (HAM) only releases to 2.4 GHz after ~3.4 us of
    # sustained activity; warming during the input DMA makes the
    # projection and the first attention row-tiles run at full rate.
    warm_f32 = const.tile([128, BANK], F32)
    nc.vector.memset(warm_f32, 0.0)
    warm = const.tile([128, BANK], MM_DT)
    nc.vector.tensor_copy(out=warm, in_=warm_f32)
    wps = psum.tile([128, ACT_CHUNK], F32, tag="ps")
    for _ in range(10):
        nc.tensor.matmul(
            wps[:, 0:BANK], warm[:, 0:128], warm, start=True, stop=True
        )

    ident = const.tile([128, 128], F32)
    make_identity(nc, ident)

    # pull the exp table load off the first real activation
    tbl = small.tile([128, 1], F32, tag="tbl")
    nc.scalar.activation(
        out=tbl, in_=warm_f32[:, 0:1], func=mybir.ActivationFunctionType.Exp
    )

    # ---- transpose W on PE -> wt_sb[c_sub, c_tile, o] (contraction c on partitions)
    wt_sb = const.tile([128, 2, 2 * D], PROJ_DT)
    for t in range(2):          # output-channel tile (q half / k half)
        for ct in range(2):     # input-channel tile
            ps = psum.tile([128, ACT_CHUNK], F32, tag="ps")
            nc.tensor.transpose(
                ps[:, 0:128], w_sb[:, t, ct * 128:(ct + 1) * 128], ident
            )
            nc.vector.tensor_copy(
                out=wt_sb[:, ct, t * 128:(t + 1) * 128], in_=ps[:, 0:128]
            )

    q_sb = data.tile([128, HW], MM_DT)
    k_sb = data.tile([128, HW], MM_DT)

    def proj_chunk(t, dst, a, x_half, banks=None):
        """Project output-channel half t for column chunk a; x_half is
        [128, chunk, c_tile, X_CHUNK] holding x columns
        [a*ACT_CHUNK, (a+1)*ACT_CHUNK). banks selects a contiguous
        subset of the four 512-wide banks; the projected span leaves
        PSUM in a single batched DVE copy."""
        ps = psum.tile([128, ACT_CHUNK], F32, tag="ps")
        bank_list = list(banks) if banks is not None else list(
            range(ACT_CHUNK // BANK))
        for jj in bank_list:
            lo = slice(jj * BANK, (jj + 1) * BANK)
            ch = (jj * BANK) // X_CHUNK
            off = (jj * BANK) % X_CHUNK
            nc.tensor.matmul(
                ps[:, lo], wt_sb[:, 0, t * 128:(t + 1) * 128],
                x_half[:, ch, 0, off:off + BANK], start=True, stop=False,
            )
            nc.tensor.matmul(
                ps[:, lo], wt_sb[:, 1, t * 128:(t + 1) * 128],
                x_half[:, ch, 1, off:off + BANK], start=False, stop=True,
            )
        b0, b1 = bank_list[0], bank_list[-1] + 1
        nc.vector.tensor_copy(
            out=dst[:, a * ACT_CHUNK + b0 * BANK:a * ACT_CHUNK + b1 * BANK],
            in_=ps[:, b0 * BANK:b1 * BANK],
        )

    outp = None

    def sim_chunk(lhs, out_row, lo_col, n_col, accum, ps=None, ps_lo=0):
        """n_col-wide slice of one attention row: matmuls + fused exp.
        A shared psum tile may be passed in (with bank offset ps_lo) so
        two sub-chunks can ping-pong inside one 4-bank tile."""
        if ps is None:
            ps = psum.tile([128, ACT_CHUNK], F32, tag="ps")
        for jj in range(n_col // BANK):
            sl = slice(lo_col + jj * BANK, lo_col + (jj + 1) * BANK)
            nc.tensor.matmul(
                ps[:, ps_lo + jj * BANK:ps_lo + (jj + 1) * BANK], lhs,
                k_sb[:, sl], start=True, stop=True,
            )
        nc.scalar.activation(
            out=out_row[:, lo_col:lo_col + n_col],
            in_=ps[:, ps_lo:ps_lo + n_col],
            func=mybir.ActivationFunctionType.Exp,
            scale=SCALE,
            accum_out=accum,
        )

    def finish_tile(out_grp, t, i, sums, split_dma):
        """Row sum -> reciprocal -> scale -> ship for one row-tile."""
        rsum = small.tile([128, 1], F32, tag="rsum")
        nc.vector.tensor_reduce(
            out=rsum, in_=sums, axis=mybir.AxisListType.X,
            op=mybir.AluOpType.add,
        )
        recip = small.tile([128, 1], F32, tag="recip")
        nc.vector.reciprocal(out=recip, in_=rsum)
        if split_dma:
            # normalize and ship each half-row as soon as it is scaled
            # (0.5 MB transfers) to shorten the pipeline head/tail
            for a in range(N_ACT):
                sl = slice(a * ACT_CHUNK, (a + 1) * ACT_CHUNK)
                nc.vector.tensor_scalar_mul(
                    out=out_grp[:, t, sl], in0=out_grp[:, t, sl],
                    scalar1=recip,
                )
                nc.sync.dma_start(
                    out=out_ap[i * 128:(i + 1) * 128, sl],
                    in_=out_grp[:, t, sl],
                )
        else:
            nc.vector.tensor_scalar_mul(
                out=out_grp[:, t, :], in0=out_grp[:, t, :], scalar1=recip
            )
            nc.sync.dma_start(
                out=out_ap[i * 128:(i + 1) * 128, :], in_=out_grp[:, t, :]
            )

    def emit_group(g, split_dma=False):
        out_grp = outp.tile([128, GRP, HW], OUT_DT, tag="out")
        for t in range(GRP):
            i = g * GRP + t
            lhs = q_sb[:, i * 128:(i + 1) * 128]
            sums = small.tile([128, N_ACT], F32, tag="sums")
            for a in range(N_ACT):
                sim_chunk(lhs, out_grp[:, t], a * ACT_CHUNK, ACT_CHUNK,
                          sums[:, a:a + 1])
            finish_tile(out_grp, t, i, sums, split_dma)

    def emit_head():
        """Row-tiles 0-3 (groups 0-1), restructured so the exp stream
        starts before x has fully landed: all chunk-0 ACTs first (they
        only need k[0:2048], i.e. the first half of x), with tile 0's
        chunk split into 1024-wide pieces that chase the first k bank
        copies; k's second half is projected mid-stream, then the
        deferred chunk-1 ACTs run."""
        g0 = outp.tile([128, GRP, HW], OUT_DT, tag="out")
        g1 = outp.tile([128, GRP, HW], OUT_DT, tag="out")
        tiles = [(g0, 0), (g0, 1), (g1, 0), (g1, 1)]
        sums_t = [small.tile([128, 3 if i == 0 else N_ACT], F32, tag="sums",
                             name=f"sums{i}") for i in range(4)]
        lhs = [q_sb[:, i * 128:(i + 1) * 128] for i in range(4)]
        # tile 0 chunk 0 as 2x1024 inside one psum tile
        ps0 = psum.tile([128, ACT_CHUNK], F32, tag="ps")
        sim_chunk(lhs[0], g0[:, 0], 0, 1024, sums_t[0][:, 0:1],
                  ps=ps0, ps_lo=0)
        sim_chunk(lhs[0], g0[:, 0], 1024, 1024, sums_t[0][:, 1:2],
                  ps=ps0, ps_lo=1024)
        for i in range(1, 4):
            og, t = tiles[i]
            sim_chunk(lhs[i], og[:, t], 0, ACT_CHUNK, sums_t[i][:, 0:1])
        # k cols 2048:4096 -- x has fully landed by now
        proj_chunk(1, k_sb, 1, x1_sb)
        # q rows 512:1024 (first read by group 2)
        proj_chunk(0, q_sb, 0, x0_sb, banks=(1,))
        for i in range(4):
            og, t = tiles[i]
            c = 2 if i == 0 else 1
            sim_chunk(lhs[i], og[:, t], ACT_CHUNK, ACT_CHUNK,
                      sums_t[i][:, c:c + 1])
            finish_tile(og, t, i, sums_t[i], split_dma=(i < 2))

    # ---- projection head: q's first bank and k's first half only need
    # the first x chunks, so the exp stream starts while the rest of x
    # is still in flight; emit_head projects k's second half mid-stream.
    # Remaining q banks trickle in one 512-wide bank at a time, each at
    # least two groups ahead of their first reader.
    proj_chunk(0, q_sb, 0, x0_sb, banks=(0,))     # q rows 0:512 (tiles 0-3)
    proj_chunk(1, k_sb, 0, x0_sb, banks=(0, 1))   # k cols 0:1024
    proj_chunk(1, k_sb, 0, x0_sb, banks=(2, 3))   # k cols 1024:2048

    outp = ctx.enter_context(tc.tile_pool(name="outp", bufs=OUT_BUFS))
    emit_head()                                   # groups 0-1
    proj_chunk(0, q_sb, 0, x0_sb, banks=(2,))     # rows 1024:1536 (grps 4-5)
    emit_group(2)
    proj_chunk(0, q_sb, 0, x0_sb, banks=(3,))     # rows 1536:2048 (grps 6-7)
    emit_group(3)
    for g in range(4, N_GRP // 2):
        # q chunk 1 (row-tiles 16-31), one bank several groups ahead
        proj_chunk(0, q_sb, 1, x1_sb, banks=(g - 4,))
        emit_group(g)
    for g in range(N_GRP // 2, N_GRP - 1):
        emit_group(g)
    # last group ships per normalized half-row so the tail after the
    # final exp is one 0.5 MB transfer, not a 2 MB one
    emit_group(N_GRP - 1, split_dma=True)


_built = None


def _get_nc():
    global _built
    if _built is None:
        nc = bacc.Bacc("TRN2", target_bir_lowering=False, debug=False)
        x = nc.dram_tensor("x", [C, HW], F32, kind="ExternalInput").ap()
        w = nc.dram_tensor("w", [2 * D, C], F32, kind="ExternalInput").ap()
        out = nc.dram_tensor("out", [HW, HW], OUT_DT, kind="ExternalOutput").ap()
        with tile.TileContext(nc) as tc:
            with ExitStack() as ctx:
                _emit(ctx, tc, out, x, w)
        nc.compile()
        _built = nc
    return _built


def kernel(x: np.ndarray, W: np.ndarray) -> np.ndarray:
    nc = _get_nc()
    x = np.asarray(x, dtype=np.float32)
    W = np.ascontiguousarray(np.asarray(W, dtype=np.float32))
    in_maps = [
        {"x": np.ascontiguousarray(x[b].reshape(C, HW)), "w": W} for b in range(B)
    ]
    res = run_bass_kernel_spmd(nc, in_maps, core_ids=list(range(N_CORES)))
    out = np.stack([res.results[b]["out"] for b in range(B)]).astype(np.float32)
    return out[:, None]

